# revision 74
# baseline (speedup 1.0000x reference)
"""Trainium2 Bass kernel for nn_GATt_to_R_78950088835242 (GNN message passing).

Math: with rel_size = arange(E), x_res2[rel_size] is the identity, and the
per-relation softmax weights alpha sum to 1 within each segment, so
    x_type[rel] == x_res2 == M2[rel],
where M2 = concat(mean_h, mean_t) @ W_sr1 + b_sr1 and mean_h/mean_t are the
per-relation means of s_t[src]/s_t[dst].  Further, the t_c1 projection
commutes with the segment mean:  mean_h = mean(x_e[src]) @ W_tc1 + b_tc1.
So the output is
    out[e] = [ x_res1[e] + (rho[r] * (A_h^T Vh + A_t^T Vt)[r] + b_eff) |
               rho[r] * (A_h^T W1)[r] + b_tc1 |
               rho[r] * (A_t^T W1)[r] + b_tc1 ]        with r = rel[e],
where A_h[k, r] = sum_{e in segment r} x_e[src[e]][k]  (raw feature segsums),
rho[r] = 1/max(count_r, 1), Vh = W_tc1 @ W_sr1[:128], Vt = W_tc1 @ W_sr1[128:],
b_eff = b_tc1 @ (W_sr1[:128] + W_sr1[128:]) + b_sr1.

Sharding: edges are bucketed by rel // 125 so core c owns relations
[125c, 125c+125).  Every per-relation table is then <= 128 rows and lives in
SBUF/PSUM; no collectives are needed (counts and sums are exact per core).
Within each core, edges are additionally SORTED by relation, so any
2048-edge super-tile only touches a narrow (<= 32 wide) window of
consecutive relations.

Device pipeline per core (SPMD, no cross-core traffic):
  pass 1: stream the fp8 node table + fp8 incidence-count matrix (both
          compacted to the ~71% of nodes this core's edges touch) with
          row-blocked (p j) layout (8 KB contiguous per partition per DMA)
          and accumulate A = x_e^T @ [Mh | Mt] in PSUM with DoubleRow fp8
          matmuls (256-deep contraction per instruction).  Runs at the
          ~420 GB/s HBM read roofline.
  stage D: tiny matmuls fold A through the (host-folded) weight products
          into a [128, 384] bf16 table 32*[M2_nobias | mean_h | mean_t] plus
          a const row (the x32 scale keeps the fp8 outputs well clear of
          subnormals; the host divides it back out).
  windows: data-driven selector matmuls compact the table into per-super-
          tile KROWS=32-row windows, WPG=3 windows stacked per 128-partition
          group at PE-legal bases 0/32/64.  The pass-2 one-hots then only
          carry the window rows: 32 B/edge of one-hot HBM traffic instead
          of 128 B/edge for full 128-row one-hots (8.1 MB -> 2.1 MB/core).
  pass 2: per 128-edge sub-tile, gather window rows via one-hot fp8 x fp8
          DoublePixel matmuls (32-row contraction, fp8 moving table window
          -> ~1.3x PE stream rate on HW; one-hot tiles fully SBUF-resident,
          prefetched during pass 1), column-split into two PSUM streams so
          evacuation groups are large without any slot straddling a 2 KB
          PSUM bank: the 128-col M2 gathers pack 8 sub-tiles per tile (DVE
          adds 32*x_res1 (bf16) in one op per 8 -> out_a bf16), the 256-col
          mean gathers pack 4 per tile (ACT casts one op per 4 -> out_b
          fp8).  The fp8 window table costs nothing numerically: out_b is
          fp8-quantized at the output anyway and the table's contribution
          to out_a is ~0.5% of x_res1.  Host upcasts and multiplies 1/32.

Pass 2 engines are balanced near the HW limits (PE ~153 us streaming at
the ~1.2 GHz p-state, ACT ~138 us of casts, DMA ~150-170 us); alternatives
tried and measured slower on HW: transposed gathers (features on
out-partitions, 512-col one-hot streams), DoubleRow gathers, bf16 windows
(plain or DoublePixel), ACT-issued store queue, merged 384-col gathers,
moving outb casts to DVE.
"""

import math
import os
import sys
import time
import types

import numpy as np


def _ensure_ntff_hook():
    """This image's antenv lacks axon_hooks; inject a shim and register the
    ctypes NTFF profile hook so trace=True can report HW exec time."""
    if "antenv.axon_hooks" in sys.modules:
        return
    mod = types.ModuleType("antenv.axon_hooks")
    mod._hook = None

    def set_axon_ntff_profile_hook(h):
        mod._hook = h

    def get_axon_ntff_profile_hook():
        return mod._hook

    mod.set_axon_ntff_profile_hook = set_axon_ntff_profile_hook
    mod.get_axon_ntff_profile_hook = get_axon_ntff_profile_hook
    sys.modules["antenv.axon_hooks"] = mod
    try:
        from trn_agent_boot.trn_boot import _ntff_profile_via_ctypes

        hook = _ntff_profile_via_ctypes("/opt/axon/libaxon_pjrt.so")
        if hook is not None:
            mod._hook = hook
    except Exception:
        pass


_ensure_ntff_hook()

N_NODES = 100000
E_TOTAL = 500000
NUM_REL = 1000
E_HID = 256
T_HID = 128
R_HID = 128
N_CORES = 8
RPC = NUM_REL // N_CORES  # 125 relations per core
P = 128
SUPER = 16  # edges per partition per pass-2 super-tile
KROWS = 32  # compact one-hot rows per super-tile (rel window width)
WPG = 3  # windows per 128-partition group (PE operand bases 0/32/64)
NBJ = 16  # node rows per partition per pass-1 DMA
NODE_TILE = NBJ * P  # 2048
EPS = P * SUPER  # edges per super-tile (2048)
SCALE = 32.0  # fp8 table scale; divided back out on host
INV_SCALE = 1.0 / SCALE

OUT_W = 3 * R_HID  # 384


def _build_program(n_super: int, n_nsuper: int, skip_crep: bool):
    from concourse import bacc, mybir, tile

    f32 = mybir.dt.float32
    f16 = mybir.dt.float16
    bf16 = mybir.dt.bfloat16
    f8 = mybir.dt.float8e4
    AOT = mybir.AluOpType
    DR = mybir.MatmulPerfMode.DoubleRow
    DPX = mybir.MatmulPerfMode.DoublePixel

    e_pad = n_super * EPS
    n_pad = n_nsuper * NODE_TILE
    G = math.ceil(n_super / WPG)

    nc = bacc.Bacc(
        "TRN2", target_bir_lowering=False, debug=False, num_devices=N_CORES
    )

    # Segment sums as a dense matmul: A = x_e^T @ [Mh | Mt] where
    # Mcat[n, r] / Mcat[n, 128+r] count edges with (src/dst)=n, rel_local=r.
    # xm[n, 0, :] = x_e row n (fp8), xm[n, 1, :] = incidence-count row n.
    xm = nc.dram_tensor("xm", [n_pad, 2, E_HID], f8, kind="ExternalInput")
    rho_in = nc.dram_tensor("rho", [P, 1], f32, kind="ExternalInput")
    xr1 = nc.dram_tensor("xr1", [e_pad, R_HID], bf16, kind="ExternalInput")
    vh = nc.dram_tensor("vh", [E_HID, R_HID], f16, kind="ExternalInput")
    vt = nc.dram_tensor("vt", [E_HID, R_HID], f16, kind="ExternalInput")
    w1 = nc.dram_tensor("w1", [E_HID, T_HID], f16, kind="ExternalInput")
    crep = nc.dram_tensor("crep", [P, OUT_W], f32, kind="ExternalInput")
    ohtp = nc.dram_tensor(
        "ohtp", [G, WPG * KROWS, SUPER * P], f8, kind="ExternalInput"
    )
    sel = nc.dram_tensor("sel", [P, G * WPG * KROWS], f8, kind="ExternalInput")
    out_a = nc.dram_tensor("out_a", [e_pad, R_HID], bf16, kind="ExternalOutput")
    out_b = nc.dram_tensor(
        "out_b", [e_pad, 2 * T_HID], f8, kind="ExternalOutput"
    )

    with tile.TileContext(nc) as tc:
        with tc.tile_pool(name="const", bufs=1) as cp:
            # Const tiles are allocated here but their DMAs are deferred to
            # just after the first two xm loads are queued: the small const
            # transfers then hide behind the big streaming reads instead of
            # delaying pass-1's first matmul.  They are only consumed in
            # stage D.
            rho_t = cp.tile([P, 1], f32, tag="rho")
            crep_t = None
            if not skip_crep:
                crep_t = cp.tile([P, OUT_W], f32, tag="crep")
            wts = {}
            for nm in ("vh", "vt", "w1"):
                for k in range(2):
                    t_ = cp.tile([P, T_HID], f16, tag=f"{nm}{k}")
                    wts[f"{nm}{k}"] = t_

            def _load_consts():
                nc.sync.dma_start(out=rho_t[:], in_=rho_in[:])
                if crep_t is not None:
                    nc.sync.dma_start(out=crep_t[:], in_=crep[:])
                for nm, h in (("vh", vh), ("vt", vt), ("w1", w1)):
                    for k in range(2):
                        nc.sync.dma_start(
                            out=wts[f"{nm}{k}"][:], in_=h[k * P : (k + 1) * P, :]
                        )

            tabl = cp.tile([P, OUT_W], bf16, tag="tabl")  # filled in stage D

            with tc.tile_pool(name="psA", bufs=1, space="PSUM") as psA:
                A = psA.tile([P, 4 * P], f32, tag="A")

                # ---- pass 1: A = x_e^T @ [Mh | Mt], streamed over node rows.
                # (p j) layout: partition p holds NBJ consecutive rows, so each
                # partition's DMA line is one contiguous 4 KB run.  DoubleRow
                # contracts two j-slots (256 nodes) per fp8 matmul.
                with tc.tile_pool(name="p1x", bufs=6) as p1x:
                    for ns in range(n_nsuper):
                        base = ns * NODE_TILE
                        xt = p1x.tile([P, NBJ, 2, E_HID], f8, tag="xt")
                        nc.sync.dma_start(
                            out=xt[:],
                            in_=xm[base : base + NODE_TILE].rearrange(
                                "(p j) t f -> p j t f", p=P
                            ),
                        )
                        if ns == min(2, n_nsuper - 1):
                            _load_consts()
                        for jj in range(0, NBJ, 2):
                            first = ns == 0 and jj == 0
                            last = ns == n_nsuper - 1 and jj == NBJ - 2
                            # A cols [0:256] = x[:,0:128]^T @ [Mh|Mt]
                            # A cols [256:512] = x[:,128:256]^T @ [Mh|Mt]
                            for k in range(2):
                                nc.tensor.matmul(
                                    out=A[:, k * 2 * P : (k + 1) * 2 * P],
                                    lhsT=xt[:, jj : jj + 2, 0, k * P : (k + 1) * P],
                                    rhs=xt[:, jj : jj + 2, 1, :],
                                    start=first and k == 0,
                                    stop=last,
                                    perf_mode=DR,
                                    skip_group_check=True,
                                )

                # ---------------- stage D: build the table ----------------
                with tc.tile_pool(name="sd", bufs=1) as sd, \
                     tc.tile_pool(name="psD", bufs=1, space="PSUM") as psD:
                    # A layout: [Ah0 | At0 | Ah1 | At1] (feat chunk f0/f1 rows)
                    # Evacuated in halves on DVE and ACT in parallel.
                    a16 = sd.tile([P, 4, P], f16, tag="a16")
                    nc.vector.tensor_copy(
                        out=a16[:, 0:2, :], in_=A[:, 0 : 2 * P]
                    )
                    nc.scalar.copy(a16[:, 2:4, :], A[:, 2 * P :])
                    ah0, at0, ah1, at1 = (a16[:, k, :] for k in range(4))
                    S = psD.tile([P, OUT_W], f32, tag="S")
                    blocks = {
                        0: [(ah0, "vh0"), (ah1, "vh1"), (at0, "vt0"), (at1, "vt1")],
                        1: [(ah0, "w10"), (ah1, "w11")],
                        2: [(at0, "w10"), (at1, "w11")],
                    }
                    for b, lst in blocks.items():
                        for i, (a, w) in enumerate(lst):
                            nc.tensor.matmul(
                                out=S[:, b * P : (b + 1) * P],
                                lhsT=a,
                                rhs=wts[w][:],
                                start=(b == 0 and i == 0),
                                stop=(b == 2 and i == len(lst) - 1),
                                skip_group_check=True,
                            )
                    if skip_crep:
                        # Zero biases: tabl = rho * S directly (one op less
                        # in the serial pass1->pass2 chain).
                        nc.vector.tensor_scalar_mul(tabl[:], S[:], rho_t[:])
                    else:
                        ssc = sd.tile([P, OUT_W], f32, tag="ssc")
                        nc.vector.tensor_scalar_mul(ssc[:], S[:], rho_t[:])
                        nc.vector.tensor_tensor(
                            out=tabl[:], in0=ssc[:], in1=crep_t[:], op=AOT.add
                        )

            # ---- window tables: edges are host-sorted by rel, so super-tile
            # s only touches a KROWS-wide window of consecutive relations.
            # Stack WPG windows per group at PE-legal partition bases
            # (0/32/64); one selector matmul + evac materializes each group.
            # Pass-2 gathers then contract over 32 rows (KROWS/128 of the
            # one-hot HBM bytes), with no per-super-tile dependency chain.
            nwin = WPG * KROWS  # 96
            tw = cp.tile([nwin, G, OUT_W], f8, tag="tw")
            sel_t = cp.tile([P, G * nwin], f8, tag="sel")
            nc.sync.dma_start(out=sel_t[:], in_=sel[:])
            with tc.tile_pool(name="pstw", bufs=2, space="PSUM") as pstw:
                for g in range(G):
                    ptw = pstw.tile([nwin, OUT_W], f32, tag="ptw")
                    nc.tensor.matmul(
                        out=ptw[:],
                        lhsT=sel_t[:, g * nwin : (g + 1) * nwin],
                        rhs=tabl[:],
                        start=True,
                        stop=True,
                        skip_group_check=True,
                    )
                    if g % 2 == 0:
                        nc.scalar.copy(tw[:, g, :], ptw[:])
                    else:
                        nc.vector.tensor_copy(out=tw[:, g, :], in_=ptw[:])

            # ---------------- pass 2: emit output rows ----------------
            # Edge e = s*EPS + p*SUPER + j lives at (partition p, slot j) of
            # super-tile s; each partition's xr/out DMA line is contiguous.
            # PSUM groups of 8/4 sub-tiles amortize the fixed SBUF/PSUM
            # access latency of the evacuation instructions.
            with tc.tile_pool(name="p2oh", bufs=G) as p2oh, \
                 tc.tile_pool(name="p2xr", bufs=7) as p2xr, \
                 tc.tile_pool(name="p2oa", bufs=6) as p2oa, \
                 tc.tile_pool(name="p2ob", bufs=6) as p2ob, \
                 tc.tile_pool(name="ps2o", bufs=2, space="PSUM") as ps2o, \
                 tc.tile_pool(name="psa2", bufs=2, space="PSUM") as psa2:
                ohg = []
                for g in range(G):
                    oh_t = p2oh.tile([nwin, SUPER * P], f8, tag="oht")
                    nc.sync.dma_start(out=oh_t[:], in_=ohtp[g])
                    ohg.append(oh_t)
                for s in range(n_super):
                    g, k = divmod(s, WPG)
                    kb = k * KROWS
                    oh_s = ohg[g]
                    xr = p2xr.tile([P, SUPER, R_HID], bf16, tag="xr")
                    nc.sync.dma_start(
                        out=xr[:],
                        in_=xr1[s * EPS : (s + 1) * EPS].rearrange(
                            "(p j) f -> p j f", p=P
                        ),
                    )
                    outa = p2oa.tile([P, SUPER, R_HID], bf16, tag="outa")
                    outb = p2ob.tile([P, SUPER, 2 * T_HID], f8, tag="outb")
                    for gg in range(SUPER // 8):
                        opsa = psa2.tile([P, 8, P], f32, tag="opsa")
                        for half in range(2):
                            opsb = ps2o.tile([P, 4, 2 * T_HID], f32, tag="ops")
                            for q in range(4):
                                j = gg * 8 + half * 4 + q
                                nc.tensor.matmul(
                                    out=opsa[:, half * 4 + q, :],
                                    lhsT=oh_s[kb : kb + KROWS, j * P : (j + 1) * P],
                                    rhs=tw[kb : kb + KROWS, g, 0:P],
                                    start=True,
                                    stop=True,
                                    perf_mode=DPX,
                                    skip_group_check=True,
                                )
                                nc.tensor.matmul(
                                    out=opsb[:, q, :],
                                    lhsT=oh_s[kb : kb + KROWS, j * P : (j + 1) * P],
                                    rhs=tw[kb : kb + KROWS, g, P:],
                                    start=True,
                                    stop=True,
                                    perf_mode=DPX,
                                    skip_group_check=True,
                                )
                            lob = gg * 8 + half * 4
                            if gg == 1 and half == 1:
                                # Deferred below: this copy runs on DVE
                                # AFTER the x_res1 adds so it relieves ACT
                                # (the busiest evac engine) without delaying
                                # the adds that gate the out_a store.
                                deferred_b = (lob, opsb)
                            else:
                                nc.scalar.copy(
                                    outb[:, lob : lob + 4, :], opsb[:]
                                )
                        lo = gg * 8
                        hi = lo + 8
                        nc.vector.tensor_tensor(
                            out=outa[:, lo:hi, :],
                            in0=opsa[:],
                            in1=xr[:, lo:hi, :],
                            op=AOT.add,
                        )
                        if gg == 1:
                            lob, opsb_d = deferred_b
                            nc.vector.tensor_copy(
                                out=outb[:, lob : lob + 4, :], in_=opsb_d[:]
                            )
                    nc.sync.dma_start(
                        out=out_a[s * EPS : (s + 1) * EPS].rearrange(
                            "(p j) f -> p j f", p=P
                        ),
                        in_=outa[:],
                    )
                    nc.sync.dma_start(
                        out=out_b[s * EPS : (s + 1) * EPS].rearrange(
                            "(p j) f -> p j f", p=P
                        ),
                        in_=outb[:],
                    )

    nc.compile()
    return nc


def _host_prep(x_e, x_res1, W_tc1, b_tc1, W_sr1, b_sr1, edge_index, rel):
    """Bucket edges by relation range, build per-core input maps."""
    x_e = np.asarray(x_e, dtype=np.float32)
    x_res1 = np.asarray(x_res1, dtype=np.float32)
    W_tc1 = np.asarray(W_tc1, dtype=np.float32)
    b_tc1 = np.asarray(b_tc1, dtype=np.float32)
    W_sr1 = np.asarray(W_sr1, dtype=np.float32)
    b_sr1 = np.asarray(b_sr1, dtype=np.float32)
    edge_index = np.asarray(edge_index)
    rel = np.asarray(rel)

    shard_of = rel // RPC
    # Sort each core's edges by relation: pass-2 super-tiles then cover a
    # narrow window of consecutive relations (compact one-hot gathers).
    idx_per_core = []
    for c in range(N_CORES):
        ix = np.flatnonzero(shard_of == c)
        idx_per_core.append(ix[np.argsort(rel[ix], kind="stable")])
    max_edges = max(len(ix) for ix in idx_per_core)
    n_super = max(1, math.ceil(max_edges / EPS))
    e_pad = n_super * EPS
    G = math.ceil(n_super / WPG)

    src = np.ascontiguousarray(edge_index[0]).astype(np.int64)
    dst = np.ascontiguousarray(edge_index[1]).astype(np.int64)

    # Per-core node compaction: only nodes touched by this core's edges.
    used = [
        np.unique(np.concatenate([src[ix], dst[ix]])) for ix in idx_per_core
    ]
    n_used_max = max(len(u) for u in used)
    n_nsuper = max(1, math.ceil(n_used_max / NODE_TILE))
    n_pad = n_nsuper * NODE_TILE

    # Host-folded weight products (constant folding of the two Linears).
    vh = (W_tc1 @ W_sr1[:T_HID]).astype(np.float16)  # [256, 128]
    vt = (W_tc1 @ W_sr1[T_HID:]).astype(np.float16)  # [256, 128]
    w1 = W_tc1.astype(np.float16)  # [256, 128]
    b_eff = b_tc1 @ (W_sr1[:T_HID] + W_sr1[T_HID:]) + b_sr1  # [128]
    const_row = np.concatenate([b_eff, b_tc1, b_tc1]).astype(np.float32)  # [384]
    crep = np.broadcast_to(const_row * SCALE, (P, OUT_W)).astype(np.float32).copy()

    import ml_dtypes

    f8 = ml_dtypes.float8_e4m3
    bf16 = ml_dtypes.bfloat16
    x8full = x_e.astype(f8)
    consts = dict(vh=vh, vt=vt, w1=w1, crep=crep)

    in_maps = []
    for c in range(N_CORES):
        ix = idx_per_core[c]
        n = len(ix)
        u = used[c]
        n_u = len(u)
        src_c = np.searchsorted(u, src[ix])
        dst_c = np.searchsorted(u, dst[ix])

        xr_c = np.zeros((e_pad, R_HID), dtype=bf16)
        rel_loc = rel[ix] - c * RPC
        xr_c[:n] = (x_res1[ix] * SCALE).astype(bf16)

        # Incidence-count matrix: mcat[n, r] = #edges(src=n, rel=r),
        # mcat[n, 128+r] = #edges(dst=n, rel=r).  Index-only preprocessing.
        # Counts stay exact in e4m3 (integers <= 16); guarded below.
        mint = np.zeros(n_pad * 2 * T_HID, dtype=np.int32)
        np.add.at(mint, src_c * E_HID + rel_loc, 1)
        np.add.at(mint, dst_c * E_HID + T_HID + rel_loc, 1)
        assert mint.max() <= 16, "fp8 count overflow"
        xm = np.zeros((n_pad, 2, E_HID), dtype=f8)
        xm[:n_u, 0] = x8full[u]
        xm[:, 1] = mint.reshape(n_pad, E_HID).astype(f8)

        cnt = np.bincount(rel_loc, minlength=P).astype(np.float64)
        rho = (SCALE / np.maximum(cnt, 1.0)).astype(np.float32)[:, None]

        # Compact transposed per-tile one-hots: super-tile s's KROWS-wide
        # rel window sits at partition rows 32*(s%WPG) of group s//WPG.
        # ohtp[s//WPG, rel-base_s+32*(s%WPG), e%EPS] = 1 iff
        # rel(edge s*EPS + (e%EPS)) == rel (columns are linear edge order,
        # matching the feature-major xr/out layout).  Pad edges hit row 125.
        # sel picks each window's table rows out of the full table.
        rel_pad = np.full(e_pad, RPC, dtype=np.int64)
        rel_pad[:n] = rel_loc
        e_ar = np.arange(e_pad)
        s_ar = e_ar // EPS
        q = e_ar % EPS
        rt = rel_pad.reshape(n_super, EPS)
        base = np.minimum(rt.min(axis=1), P - KROWS)
        assert int((rt.max(axis=1) - base + 1).max()) <= KROWS
        nwin = WPG * KROWS
        ohtp = np.zeros((G, nwin, SUPER * P), dtype=f8)
        ohtp[
            s_ar // WPG,
            rel_pad - base[s_ar] + KROWS * (s_ar % WPG),
            (q % SUPER) * P + q // SUPER,
        ] = 1.0
        sel_h = np.zeros((P, G * nwin), dtype=f8)
        t_ar = np.arange(KROWS)
        for s in range(n_super):
            sel_h[
                base[s] + t_ar,
                (s // WPG) * nwin + KROWS * (s % WPG) + t_ar,
            ] = 1.0

        m = dict(
            xm=xm,
            rho=rho,
            ohtp=ohtp,
            sel=sel_h,
            xr1=xr_c,
            **consts,
        )
        in_maps.append(m)
    skip_crep = bool(np.all(const_row == 0.0))
    return in_maps, idx_per_core, n_super, n_nsuper, e_pad, skip_crep


_prog_cache: dict[tuple, object] = {}

last_exec_time_ns = None
last_results = None


def kernel(
    x_e,
    x_res1,
    W_tc1,
    b_tc1,
    W_sr1,
    b_sr1,
    a1,
    a5,
    edge_index,
    rel,
    rel_size,
):
    global last_exec_time_ns, last_results
    from concourse.bass_utils import run_bass_kernel_spmd

    in_maps, idx_per_core, n_super, n_nsuper, e_pad, skip_crep = _host_prep(
        x_e, x_res1, W_tc1, b_tc1, W_sr1, b_sr1, edge_index, rel
    )

    key = (n_super, n_nsuper, skip_crep)
    if key not in _prog_cache:
        t0 = time.time()
        _prog_cache[key] = _build_program(n_super, n_nsuper, skip_crep)
        print(f"[kernel] built+compiled program in {time.time() - t0:.1f}s")
    nc = _prog_cache[key]

    trace = os.environ.get("KBENCH_TRACE", "1") == "1"
    t0 = time.time()
    res = run_bass_kernel_spmd(nc, in_maps, list(range(N_CORES)), trace=trace)
    print(f"[kernel] device run (incl staging) {time.time() - t0:.1f}s")
    last_exec_time_ns = getattr(res, "exec_time_ns", None)
    last_results = res

    out = np.empty((E_TOTAL, OUT_W), dtype=np.float32)
    for c in range(N_CORES):
        ix = idx_per_core[c]
        n = len(ix)
        out[ix, :R_HID] = (
            res.results[c]["out_a"][:n].astype(np.float32) * INV_SCALE
        )
        out[ix, R_HID:] = (
            res.results[c]["out_b"][:n].astype(np.float32) * INV_SCALE
        )
    return out



# revision 75
# speedup vs baseline: 1.1299x; 1.1299x over previous
"""Trainium2 Bass kernel for nn_GATt_to_R_78950088835242 (GNN message passing).

Math: with rel_size = arange(E), x_res2[rel_size] is the identity, and the
per-relation softmax weights alpha sum to 1 within each segment, so
    x_type[rel] == x_res2 == M2[rel],
where M2 = concat(mean_h, mean_t) @ W_sr1 + b_sr1 and mean_h/mean_t are the
per-relation means of s_t[src]/s_t[dst].  Further, the t_c1 projection
commutes with the segment mean:  mean_h = mean(x_e[src]) @ W_tc1 + b_tc1.
So the output is
    out[e] = [ x_res1[e] + (rho[r] * (A_h^T Vh + A_t^T Vt)[r] + b_eff) |
               rho[r] * (A_h^T W1)[r] + b_tc1 |
               rho[r] * (A_t^T W1)[r] + b_tc1 ]        with r = rel[e],
where A_h[k, r] = sum_{e in segment r} x_e[src[e]][k]  (raw feature segsums),
rho[r] = 1/max(count_r, 1), Vh = W_tc1 @ W_sr1[:128], Vt = W_tc1 @ W_sr1[128:],
b_eff = b_tc1 @ (W_sr1[:128] + W_sr1[128:]) + b_sr1.

Sharding: edges are bucketed by rel // 125 so core c owns relations
[125c, 125c+125).  Every per-relation table is then <= 128 rows and lives in
SBUF/PSUM; no collectives are needed (counts and sums are exact per core).
Within each core, edges are additionally SORTED by relation, so any
2048-edge super-tile only touches a narrow (<= 32 wide) window of
consecutive relations.

Device pipeline per core (SPMD, no cross-core traffic):
  pass 1: stream the fp8 node table + fp8 incidence-count matrix (both
          compacted to the ~71% of nodes this core's edges touch) with
          row-blocked (p j) layout (8 KB contiguous per partition per DMA)
          and accumulate A = x_e^T @ [Mh | Mt] in PSUM with DoubleRow fp8
          matmuls (256-deep contraction per instruction).  Runs at the
          ~420 GB/s HBM read roofline.
  stage D: tiny matmuls fold A through the (host-folded) weight products
          into a [128, 384] bf16 table 32*[M2_nobias | mean_h | mean_t] plus
          a const row (the x32 scale keeps the fp8 outputs well clear of
          subnormals; the host divides it back out).
  windows: data-driven selector matmuls compact the table into per-super-
          tile KROWS=32-row windows, WPG=3 windows stacked per 128-partition
          group at PE-legal bases 0/32/64.  The pass-2 one-hots then only
          carry the window rows: 32 B/edge of one-hot HBM traffic instead
          of 128 B/edge for full 128-row one-hots (8.1 MB -> 2.1 MB/core).
  pass 2: per 128-edge sub-tile, gather window rows via one-hot fp8 x fp8
          DoublePixel matmuls (32-row contraction, fp8 moving table window
          -> ~1.3x PE stream rate on HW; one-hot tiles fully SBUF-resident,
          prefetched during pass 1), column-split into two PSUM streams so
          evacuation groups are large without any slot straddling a 2 KB
          PSUM bank: the 128-col M2 gathers pack 8 sub-tiles per tile (DVE
          adds 32*x_res1 (bf16) in one op per 8 -> out_a bf16), the 256-col
          mean gathers pack 4 per tile (ACT casts one op per 4 -> out_b
          fp8).  The fp8 window table costs nothing numerically: out_b is
          fp8-quantized at the output anyway and the table's contribution
          to out_a is ~0.5% of x_res1.  Host upcasts and multiplies 1/32.

Pass 2 engines are balanced near the HW limits (PE ~153 us streaming at
the ~1.2 GHz p-state, ACT ~138 us of casts, DMA ~150-170 us); alternatives
tried and measured slower on HW: transposed gathers (features on
out-partitions, 512-col one-hot streams), DoubleRow gathers, bf16 windows
(plain or DoublePixel), ACT-issued store queue, merged 384-col gathers,
moving outb casts to DVE.
"""

import math
import os
import sys
import time
import types

import numpy as np


def _ensure_ntff_hook():
    """This image's antenv lacks axon_hooks; inject a shim and register the
    ctypes NTFF profile hook so trace=True can report HW exec time."""
    if "antenv.axon_hooks" in sys.modules:
        return
    mod = types.ModuleType("antenv.axon_hooks")
    mod._hook = None

    def set_axon_ntff_profile_hook(h):
        mod._hook = h

    def get_axon_ntff_profile_hook():
        return mod._hook

    mod.set_axon_ntff_profile_hook = set_axon_ntff_profile_hook
    mod.get_axon_ntff_profile_hook = get_axon_ntff_profile_hook
    sys.modules["antenv.axon_hooks"] = mod
    try:
        from trn_agent_boot.trn_boot import _ntff_profile_via_ctypes

        hook = _ntff_profile_via_ctypes("/opt/axon/libaxon_pjrt.so")
        if hook is not None:
            mod._hook = hook
    except Exception:
        pass


_ensure_ntff_hook()

N_NODES = 100000
E_TOTAL = 500000
NUM_REL = 1000
E_HID = 256
T_HID = 128
R_HID = 128
N_CORES = 8
RPC = NUM_REL // N_CORES  # 125 relations per core
P = 128
SUPER = 16  # edges per partition per pass-2 super-tile
KROWS = 32  # compact one-hot rows per super-tile (rel window width)
WPG = 3  # windows per 128-partition group (PE operand bases 0/32/64)
NBJ = 16  # node rows per partition per pass-1 DMA
NODE_TILE = NBJ * P  # 2048
EPS = P * SUPER  # edges per super-tile (2048)
SCALE = 32.0  # fp8 table scale; divided back out on host
INV_SCALE = 1.0 / SCALE

OUT_W = 3 * R_HID  # 384


def _build_program(n_super: int, n_nsuper: int, skip_crep: bool):
    from concourse import bacc, mybir, tile

    f32 = mybir.dt.float32
    f16 = mybir.dt.float16
    bf16 = mybir.dt.bfloat16
    f8 = mybir.dt.float8e4
    AOT = mybir.AluOpType
    DR = mybir.MatmulPerfMode.DoubleRow
    DPX = mybir.MatmulPerfMode.DoublePixel

    e_pad = n_super * EPS
    n_pad = n_nsuper * NODE_TILE
    G = math.ceil(n_super / WPG)

    nc = bacc.Bacc(
        "TRN2", target_bir_lowering=False, debug=False, num_devices=N_CORES
    )

    # Segment sums as a dense matmul: A = x_e^T @ [Mh | Mt] where
    # Mcat[n, r] / Mcat[n, 128+r] count edges with (src/dst)=n, rel_local=r.
    # xm[n, 0, :] = x_e row n (fp8), xm[n, 1, :] = incidence-count row n.
    xm = nc.dram_tensor("xm", [n_pad, 2, E_HID], f8, kind="ExternalInput")
    rho_in = nc.dram_tensor("rho", [P, 1], f32, kind="ExternalInput")
    xr1 = nc.dram_tensor("xr1", [e_pad, R_HID], bf16, kind="ExternalInput")
    vh = nc.dram_tensor("vh", [E_HID, R_HID], f16, kind="ExternalInput")
    vt = nc.dram_tensor("vt", [E_HID, R_HID], f16, kind="ExternalInput")
    w1 = nc.dram_tensor("w1", [E_HID, T_HID], f16, kind="ExternalInput")
    crep = nc.dram_tensor("crep", [P, OUT_W], f32, kind="ExternalInput")
    ohtp = nc.dram_tensor(
        "ohtp", [G, WPG * KROWS, SUPER * P], f8, kind="ExternalInput"
    )
    sel = nc.dram_tensor("sel", [P, G * WPG * KROWS], f8, kind="ExternalInput")
    out_a = nc.dram_tensor("out_a", [e_pad, R_HID], bf16, kind="ExternalOutput")
    out_b = nc.dram_tensor(
        "out_b", [e_pad, 2 * T_HID], f8, kind="ExternalOutput"
    )

    with tile.TileContext(nc) as tc:
        with tc.tile_pool(name="const", bufs=1) as cp:
            # Const tiles are allocated here but their DMAs are deferred to
            # just after the first two xm loads are queued: the small const
            # transfers then hide behind the big streaming reads instead of
            # delaying pass-1's first matmul.  They are only consumed in
            # stage D.
            rho_t = cp.tile([P, 1], f32, tag="rho")
            crep_t = None
            if not skip_crep:
                crep_t = cp.tile([P, OUT_W], f32, tag="crep")
            wts = {}
            for nm in ("vh", "vt", "w1"):
                for k in range(2):
                    t_ = cp.tile([P, T_HID], f16, tag=f"{nm}{k}")
                    wts[f"{nm}{k}"] = t_

            def _load_consts():
                nc.sync.dma_start(out=rho_t[:], in_=rho_in[:])
                if crep_t is not None:
                    nc.sync.dma_start(out=crep_t[:], in_=crep[:])
                for nm, h in (("vh", vh), ("vt", vt), ("w1", w1)):
                    for k in range(2):
                        nc.sync.dma_start(
                            out=wts[f"{nm}{k}"][:], in_=h[k * P : (k + 1) * P, :]
                        )

            tabl = cp.tile([P, OUT_W], bf16, tag="tabl")  # filled in stage D

            with tc.tile_pool(name="psA", bufs=1, space="PSUM") as psA:
                A = psA.tile([P, 4 * P], f32, tag="A")

                # ---- pass 1: A = x_e^T @ [Mh | Mt], streamed over node rows.
                # (p j) layout: partition p holds NBJ consecutive rows, so each
                # partition's DMA line is one contiguous 4 KB run.  DoubleRow
                # contracts two j-slots (256 nodes) per fp8 matmul.
                with tc.tile_pool(name="p1x", bufs=6) as p1x:
                    for ns in range(n_nsuper):
                        base = ns * NODE_TILE
                        xt = p1x.tile([P, NBJ, 2, E_HID], f8, tag="xt")
                        nc.sync.dma_start(
                            out=xt[:],
                            in_=xm[base : base + NODE_TILE].rearrange(
                                "(p j) t f -> p j t f", p=P
                            ),
                        )
                        if ns == min(2, n_nsuper - 1):
                            _load_consts()
                        for jj in range(0, NBJ, 2):
                            first = ns == 0 and jj == 0
                            last = ns == n_nsuper - 1 and jj == NBJ - 2
                            # A cols [0:256] = x[:,0:128]^T @ [Mh|Mt]
                            # A cols [256:512] = x[:,128:256]^T @ [Mh|Mt]
                            for k in range(2):
                                nc.tensor.matmul(
                                    out=A[:, k * 2 * P : (k + 1) * 2 * P],
                                    lhsT=xt[:, jj : jj + 2, 0, k * P : (k + 1) * P],
                                    rhs=xt[:, jj : jj + 2, 1, :],
                                    start=first and k == 0,
                                    stop=last,
                                    perf_mode=DR,
                                    skip_group_check=True,
                                )

                # ---------------- stage D: build the table ----------------
                with tc.tile_pool(name="sd", bufs=1) as sd, \
                     tc.tile_pool(name="psD", bufs=1, space="PSUM") as psD:
                    # A layout: [Ah0 | At0 | Ah1 | At1] (feat chunk f0/f1 rows)
                    # Evacuated in halves on DVE and ACT in parallel.
                    a16 = sd.tile([P, 4, P], f16, tag="a16")
                    nc.vector.tensor_copy(
                        out=a16[:, 0:2, :], in_=A[:, 0 : 2 * P]
                    )
                    nc.scalar.copy(a16[:, 2:4, :], A[:, 2 * P :])
                    ah0, at0, ah1, at1 = (a16[:, k, :] for k in range(4))
                    S = psD.tile([P, OUT_W], f32, tag="S")
                    blocks = {
                        0: [(ah0, "vh0"), (ah1, "vh1"), (at0, "vt0"), (at1, "vt1")],
                        1: [(ah0, "w10"), (ah1, "w11")],
                        2: [(at0, "w10"), (at1, "w11")],
                    }
                    for b, lst in blocks.items():
                        for i, (a, w) in enumerate(lst):
                            nc.tensor.matmul(
                                out=S[:, b * P : (b + 1) * P],
                                lhsT=a,
                                rhs=wts[w][:],
                                start=(b == 0 and i == 0),
                                stop=(b == 2 and i == len(lst) - 1),
                                skip_group_check=True,
                            )
                    if skip_crep:
                        # Zero biases: tabl = rho * S directly (one op less
                        # in the serial pass1->pass2 chain).
                        nc.vector.tensor_scalar_mul(tabl[:], S[:], rho_t[:])
                    else:
                        ssc = sd.tile([P, OUT_W], f32, tag="ssc")
                        nc.vector.tensor_scalar_mul(ssc[:], S[:], rho_t[:])
                        nc.vector.tensor_tensor(
                            out=tabl[:], in0=ssc[:], in1=crep_t[:], op=AOT.add
                        )

            # ---- window tables: edges are host-sorted by rel, so super-tile
            # s only touches a KROWS-wide window of consecutive relations.
            # Stack WPG windows per group at PE-legal partition bases
            # (0/32/64); one selector matmul + evac materializes each group.
            # Pass-2 gathers then contract over 32 rows (KROWS/128 of the
            # one-hot HBM bytes), with no per-super-tile dependency chain.
            nwin = WPG * KROWS  # 96
            tw = cp.tile([nwin, G, OUT_W], f8, tag="tw")
            sel_t = cp.tile([P, G * nwin], f8, tag="sel")
            nc.sync.dma_start(out=sel_t[:], in_=sel[:])
            with tc.tile_pool(name="pstw", bufs=2, space="PSUM") as pstw:
                for g in range(G):
                    ptw = pstw.tile([nwin, OUT_W], f32, tag="ptw")
                    nc.tensor.matmul(
                        out=ptw[:],
                        lhsT=sel_t[:, g * nwin : (g + 1) * nwin],
                        rhs=tabl[:],
                        start=True,
                        stop=True,
                        skip_group_check=True,
                    )
                    if g % 2 == 0:
                        nc.scalar.copy(tw[:, g, :], ptw[:])
                    else:
                        nc.vector.tensor_copy(out=tw[:, g, :], in_=ptw[:])

            # ---------------- pass 2: emit output rows ----------------
            # Edge e = s*EPS + p*SUPER + j lives at (partition p, slot j) of
            # super-tile s; each partition's xr/out DMA line is contiguous.
            # PSUM groups of 8/4 sub-tiles amortize the fixed SBUF/PSUM
            # access latency of the evacuation instructions.
            with tc.tile_pool(name="p2oh", bufs=G) as p2oh, \
                 tc.tile_pool(name="p2xr", bufs=7) as p2xr, \
                 tc.tile_pool(name="p2oa", bufs=6) as p2oa, \
                 tc.tile_pool(name="p2ob", bufs=6) as p2ob, \
                 tc.tile_pool(name="ps2o", bufs=2, space="PSUM") as ps2o, \
                 tc.tile_pool(name="psa2", bufs=2, space="PSUM") as psa2:
                ohg = []
                for g in range(G):
                    oh_t = p2oh.tile([nwin, SUPER * P], f8, tag="oht")
                    nc.sync.dma_start(out=oh_t[:], in_=ohtp[g])
                    ohg.append(oh_t)
                for s in range(n_super):
                    g, k = divmod(s, WPG)
                    kb = k * KROWS
                    oh_s = ohg[g]
                    xr = p2xr.tile([P, SUPER, R_HID], bf16, tag="xr")
                    nc.sync.dma_start(
                        out=xr[:],
                        in_=xr1[s * EPS : (s + 1) * EPS].rearrange(
                            "(p j) f -> p j f", p=P
                        ),
                    )
                    outa = p2oa.tile([P, SUPER, R_HID], bf16, tag="outa")
                    outb = p2ob.tile([P, SUPER, 2 * T_HID], f8, tag="outb")
                    for gg in range(SUPER // 8):
                        opsa = psa2.tile([P, 8, P], f32, tag="opsa")
                        for half in range(2):
                            opsb = ps2o.tile([P, 4, 2 * T_HID], f32, tag="ops")
                            for q in range(4):
                                j = gg * 8 + half * 4 + q
                                nc.tensor.matmul(
                                    out=opsa[:, half * 4 + q, :],
                                    lhsT=oh_s[kb : kb + KROWS, j * P : (j + 1) * P],
                                    rhs=tw[kb : kb + KROWS, g, 0:P],
                                    start=True,
                                    stop=True,
                                    perf_mode=DPX,
                                    skip_group_check=True,
                                )
                                nc.tensor.matmul(
                                    out=opsb[:, q, :],
                                    lhsT=oh_s[kb : kb + KROWS, j * P : (j + 1) * P],
                                    rhs=tw[kb : kb + KROWS, g, P:],
                                    start=True,
                                    stop=True,
                                    perf_mode=DPX,
                                    skip_group_check=True,
                                )
                            lob = gg * 8 + half * 4
                            nc.scalar.copy(outb[:, lob : lob + 4, :], opsb[:])
                        lo = gg * 8
                        hi = lo + 8
                        nc.vector.tensor_tensor(
                            out=outa[:, lo:hi, :],
                            in0=opsa[:],
                            in1=xr[:, lo:hi, :],
                            op=AOT.add,
                        )
                    nc.sync.dma_start(
                        out=out_a[s * EPS : (s + 1) * EPS].rearrange(
                            "(p j) f -> p j f", p=P
                        ),
                        in_=outa[:],
                    )
                    nc.sync.dma_start(
                        out=out_b[s * EPS : (s + 1) * EPS].rearrange(
                            "(p j) f -> p j f", p=P
                        ),
                        in_=outb[:],
                    )

    nc.compile()
    return nc


def _host_prep(x_e, x_res1, W_tc1, b_tc1, W_sr1, b_sr1, edge_index, rel):
    """Bucket edges by relation range, build per-core input maps."""
    x_e = np.asarray(x_e, dtype=np.float32)
    x_res1 = np.asarray(x_res1, dtype=np.float32)
    W_tc1 = np.asarray(W_tc1, dtype=np.float32)
    b_tc1 = np.asarray(b_tc1, dtype=np.float32)
    W_sr1 = np.asarray(W_sr1, dtype=np.float32)
    b_sr1 = np.asarray(b_sr1, dtype=np.float32)
    edge_index = np.asarray(edge_index)
    rel = np.asarray(rel)

    shard_of = rel // RPC
    # Sort each core's edges by relation: pass-2 super-tiles then cover a
    # narrow window of consecutive relations (compact one-hot gathers).
    idx_per_core = []
    for c in range(N_CORES):
        ix = np.flatnonzero(shard_of == c)
        idx_per_core.append(ix[np.argsort(rel[ix], kind="stable")])
    max_edges = max(len(ix) for ix in idx_per_core)
    n_super = max(1, math.ceil(max_edges / EPS))
    e_pad = n_super * EPS
    G = math.ceil(n_super / WPG)

    src = np.ascontiguousarray(edge_index[0]).astype(np.int64)
    dst = np.ascontiguousarray(edge_index[1]).astype(np.int64)

    # Per-core node compaction: only nodes touched by this core's edges.
    used = [
        np.unique(np.concatenate([src[ix], dst[ix]])) for ix in idx_per_core
    ]
    n_used_max = max(len(u) for u in used)
    n_nsuper = max(1, math.ceil(n_used_max / NODE_TILE))
    n_pad = n_nsuper * NODE_TILE

    # Host-folded weight products (constant folding of the two Linears).
    vh = (W_tc1 @ W_sr1[:T_HID]).astype(np.float16)  # [256, 128]
    vt = (W_tc1 @ W_sr1[T_HID:]).astype(np.float16)  # [256, 128]
    w1 = W_tc1.astype(np.float16)  # [256, 128]
    b_eff = b_tc1 @ (W_sr1[:T_HID] + W_sr1[T_HID:]) + b_sr1  # [128]
    const_row = np.concatenate([b_eff, b_tc1, b_tc1]).astype(np.float32)  # [384]
    crep = np.broadcast_to(const_row * SCALE, (P, OUT_W)).astype(np.float32).copy()

    import ml_dtypes

    f8 = ml_dtypes.float8_e4m3
    bf16 = ml_dtypes.bfloat16
    x8full = x_e.astype(f8)
    consts = dict(vh=vh, vt=vt, w1=w1, crep=crep)

    in_maps = []
    for c in range(N_CORES):
        ix = idx_per_core[c]
        n = len(ix)
        u = used[c]
        n_u = len(u)
        src_c = np.searchsorted(u, src[ix])
        dst_c = np.searchsorted(u, dst[ix])

        xr_c = np.zeros((e_pad, R_HID), dtype=bf16)
        rel_loc = rel[ix] - c * RPC
        xr_c[:n] = (x_res1[ix] * SCALE).astype(bf16)

        # Incidence-count matrix: mcat[n, r] = #edges(src=n, rel=r),
        # mcat[n, 128+r] = #edges(dst=n, rel=r).  Index-only preprocessing.
        # Counts stay exact in e4m3 (integers <= 16); guarded below.
        mint = np.zeros(n_pad * 2 * T_HID, dtype=np.int32)
        np.add.at(mint, src_c * E_HID + rel_loc, 1)
        np.add.at(mint, dst_c * E_HID + T_HID + rel_loc, 1)
        assert mint.max() <= 16, "fp8 count overflow"
        xm = np.zeros((n_pad, 2, E_HID), dtype=f8)
        xm[:n_u, 0] = x8full[u]
        xm[:, 1] = mint.reshape(n_pad, E_HID).astype(f8)

        cnt = np.bincount(rel_loc, minlength=P).astype(np.float64)
        rho = (SCALE / np.maximum(cnt, 1.0)).astype(np.float32)[:, None]

        # Compact transposed per-tile one-hots: super-tile s's KROWS-wide
        # rel window sits at partition rows 32*(s%WPG) of group s//WPG.
        # ohtp[s//WPG, rel-base_s+32*(s%WPG), e%EPS] = 1 iff
        # rel(edge s*EPS + (e%EPS)) == rel (columns are linear edge order,
        # matching the feature-major xr/out layout).  Pad edges hit row 125.
        # sel picks each window's table rows out of the full table.
        rel_pad = np.full(e_pad, RPC, dtype=np.int64)
        rel_pad[:n] = rel_loc
        e_ar = np.arange(e_pad)
        s_ar = e_ar // EPS
        q = e_ar % EPS
        rt = rel_pad.reshape(n_super, EPS)
        base = np.minimum(rt.min(axis=1), P - KROWS)
        assert int((rt.max(axis=1) - base + 1).max()) <= KROWS
        nwin = WPG * KROWS
        ohtp = np.zeros((G, nwin, SUPER * P), dtype=f8)
        ohtp[
            s_ar // WPG,
            rel_pad - base[s_ar] + KROWS * (s_ar % WPG),
            (q % SUPER) * P + q // SUPER,
        ] = 1.0
        sel_h = np.zeros((P, G * nwin), dtype=f8)
        t_ar = np.arange(KROWS)
        for s in range(n_super):
            sel_h[
                base[s] + t_ar,
                (s // WPG) * nwin + KROWS * (s % WPG) + t_ar,
            ] = 1.0

        m = dict(
            xm=xm,
            rho=rho,
            ohtp=ohtp,
            sel=sel_h,
            xr1=xr_c,
            **consts,
        )
        in_maps.append(m)
    skip_crep = bool(np.all(const_row == 0.0))
    return in_maps, idx_per_core, n_super, n_nsuper, e_pad, skip_crep


_prog_cache: dict[tuple, object] = {}

last_exec_time_ns = None
last_results = None


def kernel(
    x_e,
    x_res1,
    W_tc1,
    b_tc1,
    W_sr1,
    b_sr1,
    a1,
    a5,
    edge_index,
    rel,
    rel_size,
):
    global last_exec_time_ns, last_results
    from concourse.bass_utils import run_bass_kernel_spmd

    in_maps, idx_per_core, n_super, n_nsuper, e_pad, skip_crep = _host_prep(
        x_e, x_res1, W_tc1, b_tc1, W_sr1, b_sr1, edge_index, rel
    )

    key = (n_super, n_nsuper, skip_crep)
    if key not in _prog_cache:
        t0 = time.time()
        _prog_cache[key] = _build_program(n_super, n_nsuper, skip_crep)
        print(f"[kernel] built+compiled program in {time.time() - t0:.1f}s")
    nc = _prog_cache[key]

    trace = os.environ.get("KBENCH_TRACE", "1") == "1"
    t0 = time.time()
    res = run_bass_kernel_spmd(nc, in_maps, list(range(N_CORES)), trace=trace)
    print(f"[kernel] device run (incl staging) {time.time() - t0:.1f}s")
    last_exec_time_ns = getattr(res, "exec_time_ns", None)
    last_results = res

    out = np.empty((E_TOTAL, OUT_W), dtype=np.float32)
    for c in range(N_CORES):
        ix = idx_per_core[c]
        n = len(ix)
        out[ix, :R_HID] = (
            res.results[c]["out_a"][:n].astype(np.float32) * INV_SCALE
        )
        out[ix, R_HID:] = (
            res.results[c]["out_b"][:n].astype(np.float32) * INV_SCALE
        )
    return out



# revision 77
# speedup vs baseline: 1.1332x; 1.0028x over previous
"""Trainium2 Bass kernel for nn_GATt_to_R_78950088835242 (GNN message passing).

Math: with rel_size = arange(E), x_res2[rel_size] is the identity, and the
per-relation softmax weights alpha sum to 1 within each segment, so
    x_type[rel] == x_res2 == M2[rel],
where M2 = concat(mean_h, mean_t) @ W_sr1 + b_sr1 and mean_h/mean_t are the
per-relation means of s_t[src]/s_t[dst].  Further, the t_c1 projection
commutes with the segment mean:  mean_h = mean(x_e[src]) @ W_tc1 + b_tc1.
So the output is
    out[e] = [ x_res1[e] + (rho[r] * (A_h^T Vh + A_t^T Vt)[r] + b_eff) |
               rho[r] * (A_h^T W1)[r] + b_tc1 |
               rho[r] * (A_t^T W1)[r] + b_tc1 ]        with r = rel[e],
where A_h[k, r] = sum_{e in segment r} x_e[src[e]][k]  (raw feature segsums),
rho[r] = 1/max(count_r, 1), Vh = W_tc1 @ W_sr1[:128], Vt = W_tc1 @ W_sr1[128:],
b_eff = b_tc1 @ (W_sr1[:128] + W_sr1[128:]) + b_sr1.

Sharding: edges are bucketed by rel // 125 so core c owns relations
[125c, 125c+125).  Every per-relation table is then <= 128 rows and lives in
SBUF/PSUM; no collectives are needed (counts and sums are exact per core).
Within each core, edges are additionally SORTED by relation, so any
2048-edge super-tile only touches a narrow (<= 32 wide) window of
consecutive relations.

Device pipeline per core (SPMD, no cross-core traffic):
  pass 1: stream the fp8 node table + fp8 incidence-count matrix (both
          compacted to the ~71% of nodes this core's edges touch) with
          row-blocked (p j) layout (8 KB contiguous per partition per DMA)
          and accumulate A = x_e^T @ [Mh | Mt] in PSUM with DoubleRow fp8
          matmuls (256-deep contraction per instruction).  Runs at the
          ~420 GB/s HBM read roofline.
  stage D: tiny matmuls fold A through the (host-folded) weight products
          into a [128, 384] bf16 table 32*[M2_nobias | mean_h | mean_t] plus
          a const row (the x32 scale keeps the fp8 outputs well clear of
          subnormals; the host divides it back out).
  windows: data-driven selector matmuls compact the table into per-super-
          tile KROWS=32-row windows, WPG=3 windows stacked per 128-partition
          group at PE-legal bases 0/32/64.  The pass-2 one-hots then only
          carry the window rows: 32 B/edge of one-hot HBM traffic instead
          of 128 B/edge for full 128-row one-hots (8.1 MB -> 2.1 MB/core).
  pass 2: per 128-edge sub-tile, gather window rows via one-hot fp8 x fp8
          DoublePixel matmuls (32-row contraction, fp8 moving table window
          -> ~1.3x PE stream rate on HW; one-hot tiles fully SBUF-resident,
          prefetched during pass 1), column-split into two PSUM streams so
          evacuation groups are large without any slot straddling a 2 KB
          PSUM bank: the 128-col M2 gathers pack 8 sub-tiles per tile (DVE
          adds 32*x_res1 (bf16) in one op per 8 -> out_a bf16), the 256-col
          mean gathers pack 4 per tile (ACT casts one op per 4 -> out_b
          fp8).  The fp8 window table costs nothing numerically: out_b is
          fp8-quantized at the output anyway and the table's contribution
          to out_a is ~0.5% of x_res1.  Host upcasts and multiplies 1/32.

Pass 2 engines are balanced near the HW limits (PE ~153 us streaming at
the ~1.2 GHz p-state, ACT ~138 us of casts, DMA ~150-170 us); alternatives
tried and measured slower on HW: transposed gathers (features on
out-partitions, 512-col one-hot streams), DoubleRow gathers, bf16 windows
(plain or DoublePixel), ACT-issued store queue, merged 384-col gathers,
moving outb casts to DVE.
"""

import math
import os
import sys
import time
import types

import numpy as np


def _ensure_ntff_hook():
    """This image's antenv lacks axon_hooks; inject a shim and register the
    ctypes NTFF profile hook so trace=True can report HW exec time."""
    if "antenv.axon_hooks" in sys.modules:
        return
    mod = types.ModuleType("antenv.axon_hooks")
    mod._hook = None

    def set_axon_ntff_profile_hook(h):
        mod._hook = h

    def get_axon_ntff_profile_hook():
        return mod._hook

    mod.set_axon_ntff_profile_hook = set_axon_ntff_profile_hook
    mod.get_axon_ntff_profile_hook = get_axon_ntff_profile_hook
    sys.modules["antenv.axon_hooks"] = mod
    try:
        from trn_agent_boot.trn_boot import _ntff_profile_via_ctypes

        hook = _ntff_profile_via_ctypes("/opt/axon/libaxon_pjrt.so")
        if hook is not None:
            mod._hook = hook
    except Exception:
        pass


_ensure_ntff_hook()

N_NODES = 100000
E_TOTAL = 500000
NUM_REL = 1000
E_HID = 256
T_HID = 128
R_HID = 128
N_CORES = 8
RPC = NUM_REL // N_CORES  # 125 relations per core
P = 128
SUPER = 16  # edges per partition per pass-2 super-tile
KROWS = 32  # compact one-hot rows per super-tile (rel window width)
WPG = 3  # windows per 128-partition group (PE operand bases 0/32/64)
NBJ = 16  # node rows per partition per pass-1 DMA
NODE_TILE = NBJ * P  # 2048
EPS = P * SUPER  # edges per super-tile (2048)
SCALE = 32.0  # fp8 table scale; divided back out on host
INV_SCALE = 1.0 / SCALE

OUT_W = 3 * R_HID  # 384


def _build_program(n_super: int, n_nsuper: int, skip_crep: bool):
    from concourse import bacc, mybir, tile

    f32 = mybir.dt.float32
    f16 = mybir.dt.float16
    bf16 = mybir.dt.bfloat16
    f8 = mybir.dt.float8e4
    AOT = mybir.AluOpType
    DR = mybir.MatmulPerfMode.DoubleRow
    DPX = mybir.MatmulPerfMode.DoublePixel

    e_pad = n_super * EPS
    n_pad = n_nsuper * NODE_TILE
    G = math.ceil(n_super / WPG)

    nc = bacc.Bacc(
        "TRN2", target_bir_lowering=False, debug=False, num_devices=N_CORES
    )

    # Segment sums as a dense matmul: A = x_e^T @ [Mh | Mt] where
    # Mcat[n, r] / Mcat[n, 128+r] count edges with (src/dst)=n, rel_local=r.
    # xm[n, 0, :] = x_e row n (fp8), xm[n, 1, :] = incidence-count row n.
    xm = nc.dram_tensor("xm", [n_pad, 2, E_HID], f8, kind="ExternalInput")
    rho_in = nc.dram_tensor("rho", [P, 1], f32, kind="ExternalInput")
    xr1 = nc.dram_tensor("xr1", [e_pad, R_HID], bf16, kind="ExternalInput")
    vh = nc.dram_tensor("vh", [E_HID, R_HID], f16, kind="ExternalInput")
    vt = nc.dram_tensor("vt", [E_HID, R_HID], f16, kind="ExternalInput")
    w1 = nc.dram_tensor("w1", [E_HID, T_HID], f16, kind="ExternalInput")
    crep = nc.dram_tensor("crep", [P, OUT_W], f32, kind="ExternalInput")
    ohtp = nc.dram_tensor(
        "ohtp", [G, WPG * KROWS, SUPER * P], f8, kind="ExternalInput"
    )
    sel = nc.dram_tensor("sel", [P, G * WPG * KROWS], f8, kind="ExternalInput")
    out_a = nc.dram_tensor("out_a", [e_pad, R_HID], bf16, kind="ExternalOutput")
    out_b = nc.dram_tensor(
        "out_b", [e_pad, 2 * T_HID], f8, kind="ExternalOutput"
    )

    with tile.TileContext(nc) as tc:
        with tc.tile_pool(name="const", bufs=1) as cp:
            # Const tiles are allocated here but their DMAs are deferred to
            # just after the first two xm loads are queued: the small const
            # transfers then hide behind the big streaming reads instead of
            # delaying pass-1's first matmul.  They are only consumed in
            # stage D.
            rho_t = cp.tile([P, 1], f32, tag="rho")
            crep_t = None
            if not skip_crep:
                crep_t = cp.tile([P, OUT_W], f32, tag="crep")
            wts = {}
            for nm in ("vh", "vt", "w1"):
                for k in range(2):
                    t_ = cp.tile([P, T_HID], f16, tag=f"{nm}{k}")
                    wts[f"{nm}{k}"] = t_

            def _load_consts():
                nc.sync.dma_start(out=rho_t[:], in_=rho_in[:])
                if crep_t is not None:
                    nc.sync.dma_start(out=crep_t[:], in_=crep[:])
                for nm, h in (("vh", vh), ("vt", vt), ("w1", w1)):
                    for k in range(2):
                        nc.sync.dma_start(
                            out=wts[f"{nm}{k}"][:], in_=h[k * P : (k + 1) * P, :]
                        )

            tabl = cp.tile([P, OUT_W], bf16, tag="tabl")  # filled in stage D

            with tc.tile_pool(name="psA", bufs=1, space="PSUM") as psA:
                A = psA.tile([P, 4 * P], f32, tag="A")

                # ---- pass 1: A = x_e^T @ [Mh | Mt], streamed over node rows.
                # (p j) layout: partition p holds NBJ consecutive rows, so each
                # partition's DMA line is one contiguous 4 KB run.  DoubleRow
                # contracts two j-slots (256 nodes) per fp8 matmul.
                with tc.tile_pool(name="p1x", bufs=5) as p1x:
                    for ns in range(n_nsuper):
                        base = ns * NODE_TILE
                        xt = p1x.tile([P, NBJ, 2, E_HID], f8, tag="xt")
                        nc.sync.dma_start(
                            out=xt[:],
                            in_=xm[base : base + NODE_TILE].rearrange(
                                "(p j) t f -> p j t f", p=P
                            ),
                        )
                        if ns == min(2, n_nsuper - 1):
                            _load_consts()
                        for jj in range(0, NBJ, 2):
                            first = ns == 0 and jj == 0
                            last = ns == n_nsuper - 1 and jj == NBJ - 2
                            # A cols [0:256] = x[:,0:128]^T @ [Mh|Mt]
                            # A cols [256:512] = x[:,128:256]^T @ [Mh|Mt]
                            for k in range(2):
                                nc.tensor.matmul(
                                    out=A[:, k * 2 * P : (k + 1) * 2 * P],
                                    lhsT=xt[:, jj : jj + 2, 0, k * P : (k + 1) * P],
                                    rhs=xt[:, jj : jj + 2, 1, :],
                                    start=first and k == 0,
                                    stop=last,
                                    perf_mode=DR,
                                    skip_group_check=True,
                                )

                # ---------------- stage D: build the table ----------------
                with tc.tile_pool(name="sd", bufs=1) as sd, \
                     tc.tile_pool(name="psD", bufs=1, space="PSUM") as psD:
                    # A layout: [Ah0 | At0 | Ah1 | At1] (feat chunk f0/f1 rows)
                    # Evacuated in halves on DVE and ACT in parallel.
                    a16 = sd.tile([P, 4, P], f16, tag="a16")
                    nc.vector.tensor_copy(
                        out=a16[:, 0:2, :], in_=A[:, 0 : 2 * P]
                    )
                    nc.scalar.copy(a16[:, 2:4, :], A[:, 2 * P :])
                    ah0, at0, ah1, at1 = (a16[:, k, :] for k in range(4))
                    S = psD.tile([P, OUT_W], f32, tag="S")
                    blocks = {
                        0: [(ah0, "vh0"), (ah1, "vh1"), (at0, "vt0"), (at1, "vt1")],
                        1: [(ah0, "w10"), (ah1, "w11")],
                        2: [(at0, "w10"), (at1, "w11")],
                    }
                    for b, lst in blocks.items():
                        for i, (a, w) in enumerate(lst):
                            nc.tensor.matmul(
                                out=S[:, b * P : (b + 1) * P],
                                lhsT=a,
                                rhs=wts[w][:],
                                start=(b == 0 and i == 0),
                                stop=(b == 2 and i == len(lst) - 1),
                                skip_group_check=True,
                            )
                    if skip_crep:
                        # Zero biases: tabl = rho * S directly (one op less
                        # in the serial pass1->pass2 chain).
                        nc.vector.tensor_scalar_mul(tabl[:], S[:], rho_t[:])
                    else:
                        ssc = sd.tile([P, OUT_W], f32, tag="ssc")
                        nc.vector.tensor_scalar_mul(ssc[:], S[:], rho_t[:])
                        nc.vector.tensor_tensor(
                            out=tabl[:], in0=ssc[:], in1=crep_t[:], op=AOT.add
                        )

            # ---- window tables: edges are host-sorted by rel, so super-tile
            # s only touches a KROWS-wide window of consecutive relations.
            # Stack WPG windows per group at PE-legal partition bases
            # (0/32/64); one selector matmul + evac materializes each group.
            # Pass-2 gathers then contract over 32 rows (KROWS/128 of the
            # one-hot HBM bytes), with no per-super-tile dependency chain.
            nwin = WPG * KROWS  # 96
            tw = cp.tile([nwin, G, OUT_W], f8, tag="tw")
            sel_t = cp.tile([P, G * nwin], f8, tag="sel")
            nc.sync.dma_start(out=sel_t[:], in_=sel[:])
            with tc.tile_pool(name="pstw", bufs=2, space="PSUM") as pstw:
                for g in range(G):
                    ptw = pstw.tile([nwin, OUT_W], f32, tag="ptw")
                    nc.tensor.matmul(
                        out=ptw[:],
                        lhsT=sel_t[:, g * nwin : (g + 1) * nwin],
                        rhs=tabl[:],
                        start=True,
                        stop=True,
                        skip_group_check=True,
                    )
                    if g % 2 == 0:
                        nc.scalar.copy(tw[:, g, :], ptw[:])
                    else:
                        nc.vector.tensor_copy(out=tw[:, g, :], in_=ptw[:])

            # ---------------- pass 2: emit output rows ----------------
            # Edge e = s*EPS + p*SUPER + j lives at (partition p, slot j) of
            # super-tile s; each partition's xr/out DMA line is contiguous.
            # PSUM groups of 8/4 sub-tiles amortize the fixed SBUF/PSUM
            # access latency of the evacuation instructions.
            with tc.tile_pool(name="p2oh", bufs=G) as p2oh, \
                 tc.tile_pool(name="p2xr", bufs=8) as p2xr, \
                 tc.tile_pool(name="p2oa", bufs=6) as p2oa, \
                 tc.tile_pool(name="p2ob", bufs=6) as p2ob, \
                 tc.tile_pool(name="ps2o", bufs=2, space="PSUM") as ps2o, \
                 tc.tile_pool(name="psa2", bufs=2, space="PSUM") as psa2:
                ohg = []
                for g in range(G):
                    oh_t = p2oh.tile([nwin, SUPER * P], f8, tag="oht")
                    nc.sync.dma_start(out=oh_t[:], in_=ohtp[g])
                    ohg.append(oh_t)
                for s in range(n_super):
                    g, k = divmod(s, WPG)
                    kb = k * KROWS
                    oh_s = ohg[g]
                    xr = p2xr.tile([P, SUPER, R_HID], bf16, tag="xr")
                    nc.sync.dma_start(
                        out=xr[:],
                        in_=xr1[s * EPS : (s + 1) * EPS].rearrange(
                            "(p j) f -> p j f", p=P
                        ),
                    )
                    outa = p2oa.tile([P, SUPER, R_HID], bf16, tag="outa")
                    outb = p2ob.tile([P, SUPER, 2 * T_HID], f8, tag="outb")
                    for gg in range(SUPER // 8):
                        opsa = psa2.tile([P, 8, P], f32, tag="opsa")
                        for half in range(2):
                            opsb = ps2o.tile([P, 4, 2 * T_HID], f32, tag="ops")
                            for q in range(4):
                                j = gg * 8 + half * 4 + q
                                nc.tensor.matmul(
                                    out=opsa[:, half * 4 + q, :],
                                    lhsT=oh_s[kb : kb + KROWS, j * P : (j + 1) * P],
                                    rhs=tw[kb : kb + KROWS, g, 0:P],
                                    start=True,
                                    stop=True,
                                    perf_mode=DPX,
                                    skip_group_check=True,
                                )
                                nc.tensor.matmul(
                                    out=opsb[:, q, :],
                                    lhsT=oh_s[kb : kb + KROWS, j * P : (j + 1) * P],
                                    rhs=tw[kb : kb + KROWS, g, P:],
                                    start=True,
                                    stop=True,
                                    perf_mode=DPX,
                                    skip_group_check=True,
                                )
                            lob = gg * 8 + half * 4
                            nc.scalar.copy(outb[:, lob : lob + 4, :], opsb[:])
                        lo = gg * 8
                        hi = lo + 8
                        nc.vector.tensor_tensor(
                            out=outa[:, lo:hi, :],
                            in0=opsa[:],
                            in1=xr[:, lo:hi, :],
                            op=AOT.add,
                        )
                    nc.sync.dma_start(
                        out=out_a[s * EPS : (s + 1) * EPS].rearrange(
                            "(p j) f -> p j f", p=P
                        ),
                        in_=outa[:],
                    )
                    nc.sync.dma_start(
                        out=out_b[s * EPS : (s + 1) * EPS].rearrange(
                            "(p j) f -> p j f", p=P
                        ),
                        in_=outb[:],
                    )

    nc.compile()
    return nc


def _host_prep(x_e, x_res1, W_tc1, b_tc1, W_sr1, b_sr1, edge_index, rel):
    """Bucket edges by relation range, build per-core input maps."""
    x_e = np.asarray(x_e, dtype=np.float32)
    x_res1 = np.asarray(x_res1, dtype=np.float32)
    W_tc1 = np.asarray(W_tc1, dtype=np.float32)
    b_tc1 = np.asarray(b_tc1, dtype=np.float32)
    W_sr1 = np.asarray(W_sr1, dtype=np.float32)
    b_sr1 = np.asarray(b_sr1, dtype=np.float32)
    edge_index = np.asarray(edge_index)
    rel = np.asarray(rel)

    shard_of = rel // RPC
    # Sort each core's edges by relation: pass-2 super-tiles then cover a
    # narrow window of consecutive relations (compact one-hot gathers).
    idx_per_core = []
    for c in range(N_CORES):
        ix = np.flatnonzero(shard_of == c)
        idx_per_core.append(ix[np.argsort(rel[ix], kind="stable")])
    max_edges = max(len(ix) for ix in idx_per_core)
    n_super = max(1, math.ceil(max_edges / EPS))
    e_pad = n_super * EPS
    G = math.ceil(n_super / WPG)

    src = np.ascontiguousarray(edge_index[0]).astype(np.int64)
    dst = np.ascontiguousarray(edge_index[1]).astype(np.int64)

    # Per-core node compaction: only nodes touched by this core's edges.
    used = [
        np.unique(np.concatenate([src[ix], dst[ix]])) for ix in idx_per_core
    ]
    n_used_max = max(len(u) for u in used)
    n_nsuper = max(1, math.ceil(n_used_max / NODE_TILE))
    n_pad = n_nsuper * NODE_TILE

    # Host-folded weight products (constant folding of the two Linears).
    vh = (W_tc1 @ W_sr1[:T_HID]).astype(np.float16)  # [256, 128]
    vt = (W_tc1 @ W_sr1[T_HID:]).astype(np.float16)  # [256, 128]
    w1 = W_tc1.astype(np.float16)  # [256, 128]
    b_eff = b_tc1 @ (W_sr1[:T_HID] + W_sr1[T_HID:]) + b_sr1  # [128]
    const_row = np.concatenate([b_eff, b_tc1, b_tc1]).astype(np.float32)  # [384]
    crep = np.broadcast_to(const_row * SCALE, (P, OUT_W)).astype(np.float32).copy()

    import ml_dtypes

    f8 = ml_dtypes.float8_e4m3
    bf16 = ml_dtypes.bfloat16
    x8full = x_e.astype(f8)
    consts = dict(vh=vh, vt=vt, w1=w1, crep=crep)

    in_maps = []
    for c in range(N_CORES):
        ix = idx_per_core[c]
        n = len(ix)
        u = used[c]
        n_u = len(u)
        src_c = np.searchsorted(u, src[ix])
        dst_c = np.searchsorted(u, dst[ix])

        xr_c = np.zeros((e_pad, R_HID), dtype=bf16)
        rel_loc = rel[ix] - c * RPC
        xr_c[:n] = (x_res1[ix] * SCALE).astype(bf16)

        # Incidence-count matrix: mcat[n, r] = #edges(src=n, rel=r),
        # mcat[n, 128+r] = #edges(dst=n, rel=r).  Index-only preprocessing.
        # Counts stay exact in e4m3 (integers <= 16); guarded below.
        mint = np.zeros(n_pad * 2 * T_HID, dtype=np.int32)
        np.add.at(mint, src_c * E_HID + rel_loc, 1)
        np.add.at(mint, dst_c * E_HID + T_HID + rel_loc, 1)
        assert mint.max() <= 16, "fp8 count overflow"
        xm = np.zeros((n_pad, 2, E_HID), dtype=f8)
        xm[:n_u, 0] = x8full[u]
        xm[:, 1] = mint.reshape(n_pad, E_HID).astype(f8)

        cnt = np.bincount(rel_loc, minlength=P).astype(np.float64)
        rho = (SCALE / np.maximum(cnt, 1.0)).astype(np.float32)[:, None]

        # Compact transposed per-tile one-hots: super-tile s's KROWS-wide
        # rel window sits at partition rows 32*(s%WPG) of group s//WPG.
        # ohtp[s//WPG, rel-base_s+32*(s%WPG), e%EPS] = 1 iff
        # rel(edge s*EPS + (e%EPS)) == rel (columns are linear edge order,
        # matching the feature-major xr/out layout).  Pad edges hit row 125.
        # sel picks each window's table rows out of the full table.
        rel_pad = np.full(e_pad, RPC, dtype=np.int64)
        rel_pad[:n] = rel_loc
        e_ar = np.arange(e_pad)
        s_ar = e_ar // EPS
        q = e_ar % EPS
        rt = rel_pad.reshape(n_super, EPS)
        base = np.minimum(rt.min(axis=1), P - KROWS)
        assert int((rt.max(axis=1) - base + 1).max()) <= KROWS
        nwin = WPG * KROWS
        ohtp = np.zeros((G, nwin, SUPER * P), dtype=f8)
        ohtp[
            s_ar // WPG,
            rel_pad - base[s_ar] + KROWS * (s_ar % WPG),
            (q % SUPER) * P + q // SUPER,
        ] = 1.0
        sel_h = np.zeros((P, G * nwin), dtype=f8)
        t_ar = np.arange(KROWS)
        for s in range(n_super):
            sel_h[
                base[s] + t_ar,
                (s // WPG) * nwin + KROWS * (s % WPG) + t_ar,
            ] = 1.0

        m = dict(
            xm=xm,
            rho=rho,
            ohtp=ohtp,
            sel=sel_h,
            xr1=xr_c,
            **consts,
        )
        in_maps.append(m)
    skip_crep = bool(np.all(const_row == 0.0))
    return in_maps, idx_per_core, n_super, n_nsuper, e_pad, skip_crep


_prog_cache: dict[tuple, object] = {}

last_exec_time_ns = None
last_results = None


def kernel(
    x_e,
    x_res1,
    W_tc1,
    b_tc1,
    W_sr1,
    b_sr1,
    a1,
    a5,
    edge_index,
    rel,
    rel_size,
):
    global last_exec_time_ns, last_results
    from concourse.bass_utils import run_bass_kernel_spmd

    in_maps, idx_per_core, n_super, n_nsuper, e_pad, skip_crep = _host_prep(
        x_e, x_res1, W_tc1, b_tc1, W_sr1, b_sr1, edge_index, rel
    )

    key = (n_super, n_nsuper, skip_crep)
    if key not in _prog_cache:
        t0 = time.time()
        _prog_cache[key] = _build_program(n_super, n_nsuper, skip_crep)
        print(f"[kernel] built+compiled program in {time.time() - t0:.1f}s")
    nc = _prog_cache[key]

    trace = os.environ.get("KBENCH_TRACE", "1") == "1"
    t0 = time.time()
    res = run_bass_kernel_spmd(nc, in_maps, list(range(N_CORES)), trace=trace)
    print(f"[kernel] device run (incl staging) {time.time() - t0:.1f}s")
    last_exec_time_ns = getattr(res, "exec_time_ns", None)
    last_results = res

    out = np.empty((E_TOTAL, OUT_W), dtype=np.float32)
    for c in range(N_CORES):
        ix = idx_per_core[c]
        n = len(ix)
        out[ix, :R_HID] = (
            res.results[c]["out_a"][:n].astype(np.float32) * INV_SCALE
        )
        out[ix, R_HID:] = (
            res.results[c]["out_b"][:n].astype(np.float32) * INV_SCALE
        )
    return out

